# revision 10
# baseline (speedup 1.0000x reference)
"""Trainium2 Bass kernel for nn_CandidateIndex (retrieval_knn, brute-force MIPS top-k).

Problem (hardcoded per spec): B=128 queries x D=64 dims, X=1,000,000 items,
k=100, n0=32 invalid ids per query, item_ids = arange(1, X+1).

Sharding: corpus split along X across 8 cores. Per core: stream the E-shard
HBM->SBUF in [128, 2048] paired chunks (two 64-dim item blocks stacked on the
partition axis so DMA uses all 128 partitions), fp32 matmul with transposed
queries (stationary) into PSUM [128 queries, 2048 items]. Tile 0 is processed
in four 512-wide sub-slices read directly from PSUM so the DVE starts during
the PE p-state ramp; tile 1 is staged and scanned alone; all later tiles are
staged PSUM->SBUF by the Activation engine (off the DVE critical path) in
PAIRS into one [128, 4096] buffer so a single DVE max/max_index pair covers
4096 items (halves DVE instruction count + candidate count; P(>8 of the
global top-132 in one 4096-window) ~ 1.7e-4/run). Local merge to top-40 per
query (the global
top-132 has >40 members on one core w.p. ~2e-7), invalid-id masking,
AllGather of (value, idx-lo, idx-hi) partials, final top-104 extraction
redundantly on every core. Winner positions are translated to item indices
with gpsimd local_scatter (per-partition 16-bit scatter): scatter j+1 into a
mark array at the winner positions, then scatter the 16-bit-split index
arrays through mark-1 (negative = ignored). Host: id lookup + dtype restore.

Perf (timeline cost model, core 0): 351762 ns. DVE-bound: the exact per-item
top-8 requires two full DVE passes over every score (max + max_index, ~297us
busy, no fast DVE modes exist for either op and no other engine can reduce
along the free axis), plus ~15us fixed AllGather cost + ~12us PE ramp-limited
warmup. Going materially faster requires giving up per-item exactness in the
scan (group-max retention) and re-scoring candidate groups exactly via an
item-major gather + small fp32 matmuls - see notes in the session memory.
"""
import numpy as np

B = 128
D = 64
X = 1_000_000
K = 100
N0 = 32
N_CORES = 8

PSUM_TILE = 2048
X_SHARD = 62 * PSUM_TILE        # 126,976 per core
X_PAD = X_SHARD * N_CORES       # 1,015,808
LOCAL_TOP = 40                  # 5 rounds of 8; P(core holds >40 of a query's global top-132) ~ 2e-7
FINAL_ROUNDS = 13               # 104 outputs >= 100
NEG = -1.0e30


def _build(x_shard=X_SHARD, local_top=LOCAL_TOP, final_rounds=FINAL_ROUNDS,
           n_cores=N_CORES, psum_tile=PSUM_TILE):
    import concourse.bass as bass
    from concourse import bacc
    import concourse.mybir as mybir
    import concourse.tile as tile

    n_tiles = x_shard // psum_tile
    n_chunks = n_tiles // 2
    n_cand = 8 * n_tiles
    n_rounds = local_top // 8
    n_gath = local_top * n_cores
    n_out = final_rounds * 8

    f32 = mybir.dt.float32
    u16 = mybir.dt.uint16
    i16 = mybir.dt.int16
    i32 = mybir.dt.int32
    Add = mybir.AluOpType.add
    Sub = mybir.AluOpType.subtract
    Mult = mybir.AluOpType.mult
    IsEq = mybir.AluOpType.is_equal

    nc = bacc.Bacc(num_devices=n_cores)

    q_dram = nc.dram_tensor("qt", [D, B], f32, kind="ExternalInput")
    e_dram = nc.dram_tensor("e", [D, x_shard], f32, kind="ExternalInput")
    inv_dram = nc.dram_tensor("inv", [B, N0], f32, kind="ExternalInput")
    off_dram = nc.dram_tensor("core_off", [1, 1], f32, kind="ExternalInput")
    out_s_dram = nc.dram_tensor("out_scores", [B, n_out], f32, kind="ExternalOutput")
    out_i_dram = nc.dram_tensor("out_idx", [B, n_out], f32, kind="ExternalOutput")

    part_dram = nc.dram_tensor("partial", [1, B * local_top * 3], f32)
    gath_dram = nc.dram_tensor("gathered", [1, n_cores * B * local_top * 3], f32,
                               addr_space="Shared")

    with tile.TileContext(nc) as tc:
        with (
            tc.tile_pool(name="chunks", bufs=3) as chunks,
            tc.tile_pool(name="persist", bufs=1) as persist,
            tc.tile_pool(name="small", bufs=3) as small,
            tc.tile_pool(name="stage", bufs=3) as stage,
            tc.tile_pool(name="psum", bufs=2, space="PSUM") as psum,
        ):
            def translate(pos_u16, nsrc, nw, lo_src_u16, hi_src_u16, tag):
                """Return (lo[B,nw] f32, hi[B,nw] f32): per-partition gather
                lo[q,j] = lo_src[q, pos[q,j]] via double local_scatter."""
                pos_i16 = persist.tile([B, nw], i16, name=f"{tag}_posi")
                nc.vector.tensor_copy(pos_i16[:], pos_u16[:])
                jramp = persist.tile([B, nw], u16, name=f"{tag}_jramp")
                nc.gpsimd.iota(jramp[:], pattern=[[1, nw]], base=1, channel_multiplier=0)
                marks = persist.tile([B, nsrc], u16, name=f"{tag}_marks")
                nc.gpsimd.local_scatter(marks[:], jramp[:], pos_i16[:],
                                        channels=B, num_elems=nsrc, num_idxs=nw)
                mm1 = persist.tile([B, nsrc], i16, name=f"{tag}_mm1")
                nc.vector.tensor_scalar(mm1[:], marks[:], 1.0, None, op0=Sub)
                lo_w = persist.tile([B, nw], u16, name=f"{tag}_low")
                nc.gpsimd.local_scatter(lo_w[:], lo_src_u16[:], mm1[:],
                                        channels=B, num_elems=nw, num_idxs=nsrc)
                hi_w = persist.tile([B, nw], u16, name=f"{tag}_hiw")
                nc.gpsimd.local_scatter(hi_w[:], hi_src_u16[:], mm1[:],
                                        channels=B, num_elems=nw, num_idxs=nsrc)
                lo_f = persist.tile([B, nw], f32, name=f"{tag}_lof")
                nc.vector.tensor_copy(lo_f[:], lo_w[:])
                hi_f = persist.tile([B, nw], f32, name=f"{tag}_hif")
                nc.vector.tensor_copy(hi_f[:], hi_w[:])
                return lo_f, hi_f

            # ---- one-time loads ----
            qT = persist.tile([2 * D, B], f32)
            nc.sync.dma_start(qT[:D, :], q_dram[:])
            nc.sync.dma_start(qT[D:, :], q_dram[:])
            inv_sb = persist.tile([B, N0], f32)
            nc.sync.dma_start(inv_sb[:], inv_dram[:])
            off_sb = persist.tile([1, 1], f32)
            nc.sync.dma_start(off_sb[:], off_dram[:])
            off_bc = persist.tile([B, 1], f32)
            nc.gpsimd.partition_broadcast(off_bc[:], off_sb[:], channels=B)
            # off in units of chunks (exact: off is a multiple of 2*psum_tile)
            offt_bc = persist.tile([B, 1], f32)
            nc.vector.tensor_scalar(offt_bc[:], off_bc[:], 1.0 / (2 * psum_tile), None, op0=Mult)
            invm1 = persist.tile([B, N0], f32)
            nc.vector.tensor_scalar(invm1[:], inv_sb[:], 1.0, None, op0=Sub)

            # Candidates are extracted per PAIR of tiles (= one 4096-item DMA
            # chunk) to halve DVE instruction count: both PSUM tiles of a
            # chunk are staged into one [B, 4096] SBUF buffer, then a single
            # max/max_index pair finds the chunk's top-8. Chunk 0 is special:
            # tile 0 is scanned in 4 sub-slices of 512 straight out of PSUM
            # (so the DVE starts during the PE p-state ramp; slots 0-31) and
            # tile 1 gets its own 2048-wide scan (slots 32-39). "tileno" is
            # therefore the CHUNK number; global idx = chunk*4096 + within.
            pair = 2 * psum_tile
            n_cand = 40 + (n_chunks - 1) * 8
            cand_v = persist.tile([B, n_cand], f32)
            cand_wt = persist.tile([B, n_cand], u16)   # within-chunk index
            tileno = persist.tile([B, n_cand], u16)
            nc.gpsimd.iota(tileno[:, :40], pattern=[[0, 40]],
                           base=0, channel_multiplier=0)
            nc.gpsimd.iota(tileno[:, 40:], pattern=[[1, n_chunks - 1], [0, 8]],
                           base=1, channel_multiplier=0)

            # ---- scan ----
            for c in range(n_chunks):
                ch = chunks.tile([128, psum_tile], f32)
                x0 = c * 2 * psum_tile
                nc.sync.dma_start(ch[:D, :], e_dram[:, x0:x0 + psum_tile])
                nc.sync.dma_start(ch[D:, :], e_dram[:, x0 + psum_tile:x0 + 2 * psum_tile])
                st = stage.tile([B, pair], f32)
                for h in range(2):
                    t = 2 * c + h
                    ps = psum.tile([B, psum_tile], f32)
                    for j in range(psum_tile // 512):
                        nc.tensor.matmul(
                            ps[:, j * 512:(j + 1) * 512], qT[h * D:(h + 1) * D, :],
                            ch[h * D:(h + 1) * D, j * 512:(j + 1) * 512],
                            start=True, stop=True)
                        if t == 0:
                            vs = cand_v[:, j * 8:(j + 1) * 8]
                            nc.vector.max(out=vs, in_=ps[:, j * 512:(j + 1) * 512])
                            nc.vector.max_index(
                                out=cand_wt[:, j * 8:(j + 1) * 8], in_max=vs,
                                in_values=ps[:, j * 512:(j + 1) * 512])
                            if j > 0:
                                nc.vector.tensor_scalar(
                                    cand_wt[:, j * 8:(j + 1) * 8],
                                    cand_wt[:, j * 8:(j + 1) * 8],
                                    float(j * 512), None, op0=Add)
                    if t == 0:
                        continue
                    nc.scalar.copy(st[:, h * psum_tile:(h + 1) * psum_tile], ps[:])
                    if t == 1:
                        # chunk 0's second tile: own 2048-wide scan, slots 32-39
                        vs = cand_v[:, 32:40]
                        nc.vector.max(out=vs, in_=st[:, psum_tile:])
                        nc.vector.max_index(out=cand_wt[:, 32:40], in_max=vs,
                                            in_values=st[:, psum_tile:])
                        nc.vector.tensor_scalar(cand_wt[:, 32:40], cand_wt[:, 32:40],
                                                float(psum_tile), None, op0=Add)
                    elif h == 1:
                        s0 = 40 + (c - 1) * 8
                        vs = cand_v[:, s0:s0 + 8]
                        nc.vector.max(out=vs, in_=st[:])
                        nc.vector.max_index(out=cand_wt[:, s0:s0 + 8],
                                            in_max=vs, in_values=st[:])

            # ---- local merge: top-LOCAL_TOP values ----
            work = persist.tile([B, n_cand], f32)
            v48 = persist.tile([B, local_top], f32)
            for r in range(n_rounds):
                src = cand_v if r == 0 else work
                nc.vector.max(out=v48[:, r * 8:(r + 1) * 8], in_=src[:])
                if r < n_rounds - 1:
                    nc.vector.match_replace(out=work[:], in_to_replace=v48[:, r * 8:(r + 1) * 8],
                                            in_values=src[:], imm_value=NEG)
            pos48 = persist.tile([B, local_top], u16)
            for r in range(n_rounds):
                nc.vector.max_index(out=pos48[:, r * 8:(r + 1) * 8],
                                    in_max=v48[:, r * 8:(r + 1) * 8], in_values=cand_v[:])

            lo48, hi48 = translate(pos48, n_cand, local_top, cand_wt, tileno, "t48")
            # global tile number; global index = hiG*psum_tile + lo
            hiG48 = persist.tile([B, local_top], f32)
            nc.vector.tensor_scalar(hiG48[:], hi48[:], offt_bc[:], None, op0=Add)
            ids48 = persist.tile([B, local_top], f32)   # item id - 1 = global index
            nc.vector.tensor_scalar(ids48[:], hiG48[:], float(2 * psum_tile), None, op0=Mult)
            nc.vector.tensor_tensor(out=ids48[:], in0=ids48[:], in1=lo48[:], op=Add)

            # ---- invalid masking (batched cross-compare) ----
            cross = persist.tile([B, local_top * N0], f32)
            nc.vector.tensor_tensor(
                out=cross[:].rearrange("b (j n) -> b j n", n=N0),
                in0=ids48[:].to_broadcast([B, local_top, N0]),
                in1=invm1[:].to_broadcast([B, N0, local_top]).rearrange("b n j -> b j n"),
                op=IsEq)
            hits = persist.tile([B, local_top], f32)
            nc.vector.tensor_reduce(hits[:], cross[:].rearrange("b (j n) -> b j n", n=N0),
                                    axis=mybir.AxisListType.X, op=mybir.AluOpType.max)
            vm48 = persist.tile([B, local_top], f32)
            nc.vector.tensor_scalar(vm48[:], hits[:], NEG, None, op0=Mult)
            nc.vector.tensor_tensor(out=vm48[:], in0=vm48[:], in1=v48[:], op=Add)

            # ---- all-gather (value, lo, hiG) ----
            pk = persist.tile([B, 3 * local_top], f32)
            nc.vector.tensor_copy(pk[:, :local_top], vm48[:])
            nc.vector.tensor_copy(pk[:, local_top:2 * local_top], lo48[:])
            nc.vector.tensor_copy(pk[:, 2 * local_top:], hiG48[:])
            nc.sync.dma_start(part_dram.ap().rearrange("o (b n) -> (o b) n", b=B), pk[:])
            nc.gpsimd.collective_compute(
                "AllGather",
                mybir.AluOpType.bypass,
                replica_groups=[list(range(n_cores))],
                ins=[part_dram[:]],
                outs=[gath_dram[:]],
            )
            # one DMA for all partials: gat[b, c, s*local_top+n]
            gat = persist.tile([B, n_cores * 3 * local_top], f32)
            nc.sync.dma_start(
                gat[:].rearrange("b (c m) -> b c m", c=n_cores),
                gath_dram.ap().rearrange("o (c b m) -> (o b) c m", c=n_cores, b=B))
            gat3 = gat[:].rearrange("b (c s n) -> b c s n", c=n_cores, s=3)
            v384v = gat3[:, :, 0, :]      # [B, n_cores, local_top] strided view
            lo384 = persist.tile([B, n_gath], u16)
            nc.vector.tensor_copy(lo384[:].rearrange("b (c n) -> b c n", c=n_cores),
                                  gat3[:, :, 1, :])
            hi384 = persist.tile([B, n_gath], u16)
            nc.vector.tensor_copy(hi384[:].rearrange("b (c n) -> b c n", c=n_cores),
                                  gat3[:, :, 2, :])
            v384 = persist.tile([B, n_gath], f32)
            nc.vector.tensor_copy(v384[:].rearrange("b (c n) -> b c n", c=n_cores), v384v)

            # ---- final top-104 ----
            fwork = persist.tile([B, n_gath], f32)
            outv = persist.tile([B, n_out], f32)
            for r in range(final_rounds):
                src = v384 if r == 0 else fwork
                nc.vector.max(out=outv[:, r * 8:(r + 1) * 8], in_=src[:])
                if r < final_rounds - 1:
                    nc.vector.match_replace(out=fwork[:], in_to_replace=outv[:, r * 8:(r + 1) * 8],
                                            in_values=src[:], imm_value=NEG)
            posf = persist.tile([B, n_out], u16)
            for r in range(final_rounds):
                nc.vector.max_index(out=posf[:, r * 8:(r + 1) * 8],
                                    in_max=outv[:, r * 8:(r + 1) * 8], in_values=v384[:])

            loF, hiF = translate(posf, n_gath, n_out, lo384, hi384, "tf")
            gif = persist.tile([B, n_out], f32)
            nc.vector.tensor_scalar(gif[:], hiF[:], float(2 * psum_tile), None, op0=Mult)
            nc.vector.tensor_tensor(out=gif[:], in0=gif[:], in1=loF[:], op=Add)

            nc.sync.dma_start(out_s_dram[:], outv[:])
            nc.sync.dma_start(out_i_dram[:], gif[:])

    nc.compile()
    return nc


_NC_CACHE = {}


def _get_nc():
    if "nc" not in _NC_CACHE:
        _NC_CACHE["nc"] = _build()
    return _NC_CACHE["nc"]


def make_in_maps(q, e_pad, inv, x_shard=X_SHARD, n_cores=N_CORES):
    in_maps = []
    for c in range(n_cores):
        in_maps.append({
            "qt": np.ascontiguousarray(np.asarray(q, dtype=np.float32).T),
            "e": np.ascontiguousarray(e_pad[:, c * x_shard:(c + 1) * x_shard]),
            "inv": np.ascontiguousarray(inv.astype(np.float32)),
            "core_off": np.array([[c * x_shard]], dtype=np.float32),
        })
    return in_maps


def make_in_maps_full(q, e, inv, x_shard=X_SHARD, n_cores=N_CORES):
    e_pad = np.zeros((D, X_PAD), dtype=np.float32)
    e_pad[:, : e.shape[1]] = e
    return make_in_maps(q, e_pad, inv, x_shard=x_shard, n_cores=n_cores)


def kernel(query_embeddings, item_embeddings_t, item_ids, invalid_ids, k):
    import sys
    if "/opt/trn_rl_repo" not in sys.path:
        sys.path.insert(0, "/opt/trn_rl_repo")
    from concourse.bass_utils import run_bass_kernel_spmd

    q = np.asarray(query_embeddings, dtype=np.float32)
    e = np.asarray(item_embeddings_t, dtype=np.float32)
    ids = np.asarray(item_ids)
    inv = np.asarray(invalid_ids)

    e_pad = np.zeros((D, X_PAD), dtype=np.float32)
    e_pad[:, :X] = e

    nc = _get_nc()
    import os
    trace = os.environ.get("KERNEL_TRACE") == "1"
    res = run_bass_kernel_spmd(nc, make_in_maps(q, e_pad, inv), list(range(N_CORES)),
                               trace=trace)
    _NC_CACHE["last_results"] = res
    out = res.results[0]
    idx = out["out_idx"][:, :K].astype(np.int64)
    scores = np.asarray(out["out_scores"][:, :K])
    top_ids = ids[0][idx]
    return top_ids, scores

